# revision 13
# baseline (speedup 1.0000x reference)
"""Multi-head causal attention on 8 Trainium2 NeuronCores.

Sharding: core c -> batch b = c // 4, head group g = c % 4 (4 of 16 heads).
Each core computes q/k/v for its 4 heads, causal softmax attention, and a
partial output  z_norm @ W_O[heads]  of shape [S, D].  Host sums the 4
head-group partials per batch and adds b_O (+ the exact b_V correction
sum_h b_V[h] @ W_O[h], since softmax rows sum to 1).

v5: bf16 GEMM operands (512-free matmuls stream at 216 ns; fp16 measured
259 ns), fp16 attention operands (129-free PV matmuls run full rate).
Everything SBUF-resident.  ALL projection / output-projection work is
emitted through generators that yield every 2 matmuls and run from a
double-buffered PSUM pool; the attention emitter ticks the generator
queue after each scores matmul + exp pair, interleaving independent PE
work into every exp() gap at sub-microsecond grain.  Scores waves are one
j-tile wide (single-bank slots, double-buffered) so the exp WAR chain
releases at the finest grain.  Diagonal-chunk scores matmuls are trimmed
to their valid column range.

PSUM: waves 2x[128,512] + fillers 2x[128,512] + z 2x[128,129] +
transpose 2x[128,128] = 8 banks.
"""

import sys

for _p in ("/opt/trn_rl_repo",):
    if _p not in sys.path:
        sys.path.insert(0, _p)

import numpy as np
import ml_dtypes

import concourse.bass as bass
from concourse import bacc
import concourse.mybir as mybir
import concourse.tile as tile
from concourse.bass_utils import run_bass_kernel_spmd
from concourse.masks import make_identity

F32 = mybir.dt.float32
BF16 = mybir.dt.bfloat16   # GEMM operands (512-free matmuls)
F16 = mybir.dt.float16     # attention operands (129-free PV matmuls)

B, S, D, H, E = 2, 2048, 2048, 16, 128
HL = 4          # heads per core
NCORES = 8
P = 128         # partitions
CH = 512        # free-dim chunk
S_T = S // P    # 16 seq tiles
S_C = S // CH   # 4 seq chunks
D_T = D // P    # 16 model-dim subtiles
D_C = D // CH   # 4 model-dim chunks
INV_SQRT_E = 1.0 / float(np.sqrt(E))


class Gen:
    """Resumable emission unit: advances one 'tick' (~2 matmuls) at a
    time so PE filler work can be interleaved at sub-microsecond grain."""

    def __init__(self, it):
        self.it = it
        self.done = False

    def step(self):
        if self.done:
            return False
        try:
            next(self.it)
            return True
        except StopIteration:
            self.done = True
            return False


def _trace_kernel(tc, xt, wq, wk, wv, wo, bq, bk, outp):
    nc = tc.nc
    ts = bass.ts

    xt3 = xt.rearrange("(o p) s -> p o s", p=P)            # [128, 16, 2048]
    wq3 = wq.rearrange("(o p) e -> p o e", p=P)            # [128, 16, 512]
    wk3 = wk.rearrange("(o p) e -> p o e", p=P)
    wv3 = wv.rearrange("(o p) e -> p o e", p=P)
    wo3 = wo.rearrange("(h p) d -> p h d", p=P)            # [128, 4, 2048]
    out3 = outp.rearrange("(t p) d -> t p d", p=P)         # [16, 128, 2048]

    from contextlib import ExitStack

    with ExitStack() as top:
        const_pool = top.enter_context(tc.tile_pool(name="consts", bufs=1))
        xpool = top.enter_context(tc.tile_pool(name="x", bufs=1))
        wpool = top.enter_context(tc.tile_pool(name="w", bufs=1))
        wopool = top.enter_context(tc.tile_pool(name="wo", bufs=1))
        qkpool = top.enter_context(tc.tile_pool(name="qk", bufs=2))
        vpool = top.enter_context(tc.tile_pool(name="v", bufs=1))
        ztpool = top.enter_context(tc.tile_pool(name="zt", bufs=1))
        expp_s = top.enter_context(tc.tile_pool(name="exps", bufs=1))
        expp_b = top.enter_context(tc.tile_pool(name="expb", bufs=1))
        zsb = top.enter_context(tc.tile_pool(name="zsb", bufs=2))
        recp = top.enter_context(tc.tile_pool(name="rec", bufs=2))
        ostage = top.enter_context(tc.tile_pool(name="ost", bufs=3))
        psS = top.enter_context(tc.tile_pool(name="psS", bufs=2, space="PSUM"))
        psF = top.enter_context(tc.tile_pool(name="psF", bufs=2, space="PSUM"))
        psZ = top.enter_context(tc.tile_pool(name="psZ", bufs=2, space="PSUM"))
        psT = top.enter_context(tc.tile_pool(name="psT", bufs=2, space="PSUM"))

        # ---------------- constants ----------------
        identity_f = const_pool.tile([P, P], F32)
        make_identity(nc, identity_f)
        identity = const_pool.tile([P, P], BF16)
        nc.vector.tensor_copy(identity, identity_f)

        # transposed causal triangle for the diagonal 128-block of scoresT:
        # valid iff local col >= p  (j <= i)
        dmask = const_pool.tile([P, P], F32)
        nc.gpsimd.memset(dmask, 0.0)
        nc.gpsimd.affine_select(
            out=dmask,
            in_=dmask,
            compare_op=mybir.AluOpType.is_ge,
            fill=-30000.0,
            base=0,
            pattern=[[1, P]],
            channel_multiplier=-1,
        )

        biases = const_pool.tile([P, 2, HL], F32)
        nc.gpsimd.dma_start(biases[:, 0, :], bq.rearrange("(h p) -> p h", p=P))
        nc.gpsimd.dma_start(biases[:, 1, :], bk.rearrange("(h p) -> p h", p=P))

        # ---------------- input DMAs ----------------
        x_sb = xpool.tile([P, D_T, S], BF16)
        wv_sb = wpool.tile([P, D_T, HL * E], BF16, name="wv")
        wq_sb = wpool.tile([P, D_T, HL * E], BF16, name="wq")
        wk_sb = wpool.tile([P, D_T, HL * E], BF16, name="wk")
        wo_sb = wopool.tile([P, HL, D], BF16)

        for d in range(D_T):
            nc.sync.dma_start(wv_sb[:, d, :], wv3[:, d, :])
        for d in range(D_T):
            nc.sync.dma_start(x_sb[:, d, ts(0, CH)], xt3[:, d, ts(0, CH)])
        for d in range(D_T):
            nc.sync.dma_start(wq_sb[:, d, :], wq3[:, d, :])
        for d in range(D_T):
            nc.sync.dma_start(wk_sb[:, d, :], wk3[:, d, :])
        for c in range(1, S_C):
            for d in range(D_T):
                nc.sync.dma_start(x_sb[:, d, ts(c, CH)], xt3[:, d, ts(c, CH)])
        for lh in range(HL):
            for dc in range(D_C):
                nc.sync.dma_start(wo_sb[:, lh, ts(dc, CH)], wo3[:, lh, ts(dc, CH)])

        # v natural layout [j_in_tile, h, jt, e + ones-column], fp16
        v_nat = vpool.tile([P, HL, S_T, E + 1], F16)
        for lh in range(HL):
            nc.vector.memset(v_nat[:, lh, :, E : E + 1], 1.0)

        zT = ztpool.tile([P, HL, S], BF16)

        # warm the PE clock gate during the input-DMA lead-in: ~10 us of
        # dependency-free matmuls on the identity tile so the first real
        # matmuls run at full clock
        for r in range(2):
            wps = psF.tile([P, CH], F32, name="fil")
            for _ in range(48):
                nc.tensor.matmul(
                    wps[:, :P], identity, identity, start=True, stop=True
                )

        qT = {}
        kT = {}

        # ---------------- tick-granular emission generators -------------
        def g_a(m, h, c):
            if c == 0:
                (qT if m == 0 else kT)[h] = qkpool.tile(
                    [P, S], BF16, name="qT" if m == 0 else "kT"
                )
            w_sb = wq_sb if m == 0 else wk_sb
            ps = psF.tile([P, CH], F32, name="fil")
            for d in range(D_T):
                nc.tensor.matmul(
                    ps, w_sb[:, d, ts(h, E)], x_sb[:, d, ts(c, CH)],
                    start=(d == 0), stop=(d == D_T - 1),
                )
                if d % 2 == 1 and d < D_T - 1:
                    yield
            # q: bq is pre-scaled by 1/sqrt(E) on host
            nc.vector.tensor_scalar(
                (qT if m == 0 else kT)[h][:, ts(c, CH)], ps,
                INV_SQRT_E if m == 0 else 1.0,
                biases[:, m, h, None],
                op0=mybir.AluOpType.mult, op1=mybir.AluOpType.add,
            )

        def g_v(jt):
            ps = psF.tile([P, CH], F32, name="fil")
            for d in range(D_T):
                nc.tensor.matmul(
                    ps, x_sb[:, d, ts(jt, P)], wv_sb[:, d, :],
                    start=(d == 0), stop=(d == D_T - 1),
                )
                if d % 2 == 1 and d < D_T - 1:
                    yield
            nc.vector.tensor_copy(
                v_nat[:, :, jt, :E],
                ps.rearrange("p (h e) -> p h e", h=HL),
            )

        def g_c(t, dc):
            ps = psF.tile([P, CH], F32, name="fil")
            for lh in range(HL):
                nc.tensor.matmul(
                    ps, zT[:, lh, ts(t, P)], wo_sb[:, lh, ts(dc, CH)],
                    start=(lh == 0), stop=(lh == HL - 1),
                )
                if lh == 1:
                    yield
            ot = ostage.tile([P, CH], BF16, name="ot")
            nc.vector.tensor_copy(ot, ps)
            nc.sync.dma_start(out3[t, :, ts(dc, CH)], ot)

        queue = []

        def tick(n=1):
            for _ in range(n):
                while queue and not queue[0].step():
                    queue.pop(0)

        def require(gens):
            for g in gens:
                while not g.done:
                    tick()

        # ---------------- attention head emitter ------------------------
        def b_head(h, require_by_chunk=None, on_chunk=None, tick_n=2):
            for c in range(S_C):
                if on_chunk:
                    on_chunk(c)
                # q and k of this head's chunk c are read by this chunk's
                # scores waves; finish their generators first
                require([ga[(0, h, c)], ga[(1, h, c)]])
                n_jt = S_C * c + 4
                expT = (expp_s if c < 2 else expp_b).tile(
                    [P, 8 if c < 2 else 16, CH], F16,
                    name="es" if c < 2 else "eb",
                )
                for jt in range(n_jt):
                    b = jt - S_C * c
                    # diagonal-region j-tiles: columns < b*128 are never
                    # read downstream; skip computing them
                    o = b * P if b > 0 else 0
                    sps = psS.tile([P, CH], F32, name="sps")
                    nc.tensor.matmul(
                        sps[:, o:CH],
                        kT[h][:, ts(jt, P)],
                        qT[h][:, c * CH + o : (c + 1) * CH],
                        start=True,
                        stop=True,
                    )
                    if b >= 0:
                        nc.vector.tensor_add(
                            sps[:, ts(b, P)], sps[:, ts(b, P)], dmask
                        )
                    nc.scalar.activation(
                        expT[:, jt, o:CH],
                        sps[:, o:CH],
                        mybir.ActivationFunctionType.Exp,
                    )
                    tick(tick_n)
                if require_by_chunk and c in require_by_chunk:
                    require(require_by_chunk[c])
                for a in range(S_C):  # i-tile within chunk
                    i = S_C * c + a
                    z_ps = psZ.tile([P, E + 1], F32, name="z_ps")
                    for jt in range(i + 1):
                        nc.tensor.matmul(
                            z_ps,
                            expT[:, jt, ts(a, P)],
                            v_nat[:, h, jt, :],
                            start=(jt == 0),
                            stop=(jt == i),
                        )
                    rec = recp.tile([P, 1], F32, name="rec")
                    nc.vector.reciprocal(rec, z_ps[:, E : E + 1])
                    z_sb = zsb.tile([P, E], BF16, name="z_sb")
                    nc.vector.tensor_scalar_mul(z_sb, z_ps[:, :E], rec)
                    tpz = psT.tile([P, P], BF16, name="tpz")
                    nc.tensor.transpose(tpz, z_sb, identity)
                    nc.vector.tensor_copy(zT[:, h, ts(i, P)], tpz)

        # ---------------- master emission order ----------------
        gv = {jt: Gen(g_v(jt)) for jt in range(S_T)}
        ga = {(m, h, c): Gen(g_a(m, h, c))
              for h in range(HL) for m in (0, 1) for c in range(S_C)}

        # queue order: v0-3 and head-0 q/k chunk-major first (so head 0's
        # attention can start as soon as chunk 0 is projected), then the
        # remaining v j-tiles, then q/k of heads 1-3
        queue.extend(gv[jt] for jt in range(4))
        for c in range(S_C):
            queue.append(ga[(0, 0, c)])
            queue.append(ga[(1, 0, c)])
        queue.extend(gv[jt] for jt in range(4, S_T))
        for h in (1, 2, 3):
            for c in range(S_C):
                queue.append(ga[(0, h, c)])
                queue.append(ga[(1, h, c)])

        require([gv[jt] for jt in range(4)])

        b_head(0, require_by_chunk={
            1: [gv[jt] for jt in range(4, 8)],
            2: [gv[jt] for jt in range(8, 12)],
            3: [gv[jt] for jt in range(12, 16)],
        })
        b_head(1)
        b_head(2)

        # head 3: stream output-projection groups into the queue per chunk,
        # gated to seq tiles whose zT rows are complete across all heads
        # (during chunk c, PV of head 3 is done through chunk c-1, i.e.
        # tiles t <= 4c-1)
        def h3_chunk(c):
            if c == 0:
                return
            ts_lo = {1: 0, 2: 4, 3: 8}[c]
            ts_hi = {1: 4, 2: 8, 3: 12}[c]
            for t in range(ts_lo, ts_hi):
                for dc in range(D_C):
                    queue.append(Gen(g_c(t, dc)))

        b_head(3, on_chunk=h3_chunk, tick_n=3)
        for t in range(12, S_T):
            for dc in range(D_C):
                queue.append(Gen(g_c(t, dc)))
        while queue:
            tick()


_NC_CACHE = {}
LAST_RESULTS = None


def _get_nc():
    if "nc" not in _NC_CACHE:
        nc = bacc.Bacc("TRN2", target_bir_lowering=False, debug=False)
        xt = nc.dram_tensor("xt", [D, S], BF16, kind="ExternalInput")
        wq = nc.dram_tensor("wq", [D, HL * E], BF16, kind="ExternalInput")
        wk = nc.dram_tensor("wk", [D, HL * E], BF16, kind="ExternalInput")
        wv = nc.dram_tensor("wv", [D, HL * E], BF16, kind="ExternalInput")
        wo = nc.dram_tensor("wo", [HL * E, D], BF16, kind="ExternalInput")
        bq = nc.dram_tensor("bq", [HL * E], F32, kind="ExternalInput")
        bk = nc.dram_tensor("bk", [HL * E], F32, kind="ExternalInput")
        outp = nc.dram_tensor("outp", [S, D], BF16, kind="ExternalOutput")
        with tile.TileContext(nc) as tc:
            _trace_kernel(tc, xt, wq, wk, wv, wo, bq, bk, outp)
        nc.compile()
        _NC_CACHE["nc"] = nc
    return _NC_CACHE["nc"]


def kernel(normalized_resid_pre, W_Q, W_K, W_V, W_O, b_Q, b_K, b_V, b_O):
    x = np.asarray(normalized_resid_pre, np.float32)
    W_Q = np.asarray(W_Q, np.float32)
    W_K = np.asarray(W_K, np.float32)
    W_V = np.asarray(W_V, np.float32)
    W_O = np.asarray(W_O, np.float32)
    b_Q = np.asarray(b_Q, np.float32)
    b_K = np.asarray(b_K, np.float32)
    b_V = np.asarray(b_V, np.float32)
    b_O = np.asarray(b_O, np.float32)

    nc = _get_nc()
    bf16 = ml_dtypes.bfloat16
    in_maps = []
    for core in range(NCORES):
        b, g = core // (NCORES // B), core % (NCORES // B)
        hs = range(g * HL, (g + 1) * HL)
        in_maps.append(
            {
                "xt": np.ascontiguousarray(x[b].T).astype(bf16),
                "wq": np.concatenate([W_Q[h] for h in hs], 1).astype(bf16),
                "wk": np.concatenate([W_K[h] for h in hs], 1).astype(bf16),
                "wv": np.concatenate([W_V[h] for h in hs], 1).astype(bf16),
                "wo": W_O[g * HL : (g + 1) * HL].reshape(HL * E, D).astype(bf16),
                "bq": np.ascontiguousarray(
                    b_Q[g * HL : (g + 1) * HL].reshape(-1) * np.float32(INV_SQRT_E)
                ),
                "bk": np.ascontiguousarray(b_K[g * HL : (g + 1) * HL].reshape(-1)),
            }
        )

    res = run_bass_kernel_spmd(nc, in_maps, core_ids=list(range(NCORES)))
    global LAST_RESULTS
    LAST_RESULTS = res
    out = np.zeros((B, S, D), np.float32)
    for core in range(NCORES):
        out[core // (NCORES // B)] += np.asarray(
            res.results[core]["outp"]
        ).astype(np.float32)
    # softmax rows sum to 1, so b_V contributes exactly b_V @ W_O per head
    out += (b_O + b_V.reshape(-1) @ W_O.reshape(H * E, D))[None, None, :]
    return out
